# revision 12
# baseline (speedup 1.0000x reference)
"""Cross-attention LLM block on 8 Trainium2 NeuronCores.

Sharding: core c handles batch b = c//2 and query-row half h = c%2
(2048 of the 4096 query rows of that batch), for ALL 16 heads.
K/V projections for a batch are computed redundantly by the two cores
sharing that batch (~12% extra FLOPs) so no cross-core communication
is needed; the host only slices/transposes inputs and concatenates
outputs.

v2 design (vs v1): the host pre-transposes activations to bf16
(xqT=[D,S], xkvT=[D,T]) so the device does zero PE transposes; V
lives in SBUF (no DRAM round-trip); weights use per-head-contiguous
host layouts for line-rate DMA; all PSUM evacuations ride the Scalar
engine (activation Identity/Copy, one table set) fusing bias adds, so
the Vector engine only does the softmax reciprocal+scale; exp is
batched over [128,1024] PSUM pairs. PSUM tags: M(pq,2) B(score
pairs + out pairs, 2x2 banks) C(pctx,1) D(pden,1) = 8 banks, each
evacuated within ~720ns so matmuls never stall.

Per-core dataflow (all matmuls bf16, N=512):
  phase 1: kT[dh,h,t] = sum_c wk_h_c.T @ xkvT_c   (+bk via ACT evac)
           v[t,tc,d]  = sum_c xkvT_c.T @ wv_c     (+bv rank-1 MM)
  phase 2, per s-block of 512 rows, per head:
           qT[dh,s]  = sum_c wq_h_c.T @ xqT_c     (+bq*s via ACT evac)
           psc[t,s]  = kT_ht.T @ qT   (pairs of t-chunks share a
                       2-bank PSUM tile; one exp per pair)
           e = exp(psc)               (ACT, bf16 -> SBUF)
           pden[1,s] += ones.T @ e ; pctx[dh,s] += v_ht.T @ e
           ACT: pden->d1, pctx->ctxu  (frees banks fast)
           DVE: recip(d1); GpSimd: broadcast; DVE: ctx = ctxu*rden
  out[s128,dg512] = sum_h ctx_hj.T @ wo_hg  (+bo rank-1, PSUM pairs)
"""

import math
import sys

for _p in ("/opt/trn_rl_repo",):
    if _p not in sys.path:
        sys.path.append(_p)

import numpy as np

import concourse.bass as bass
import concourse.mybir as mybir
import concourse.tile as tile
from concourse import bacc
from concourse.bass_utils import run_bass_kernel_spmd

F32 = mybir.dt.float32
BF16 = mybir.dt.bfloat16
AF = mybir.ActivationFunctionType

# full-problem dims
B, S_FULL, T_FULL, D_MODEL, NUM_HEADS = 4, 4096, 1024, 2048, 16
HEAD_DIM = 128
N_CORES = 8
S_LOC = (B * S_FULL) // N_CORES  # 2048 query rows per core


def build_program(S=S_LOC, T=T_FULL, D=D_MODEL, H=NUM_HEADS):
    """Build + compile the single-core program (SPMD across 8 cores)."""
    DH = HEAD_DIM
    NIC = D // 128          # contraction chunks
    TH = T // 2             # t-half (512 cols per K psum)
    NTC = T // 128          # t-chunks
    SB = min(512, S)        # s-block
    NSB = S // SB
    NJ = SB // 128          # 128-row subchunks per s-block
    NVG = D // 512          # v-projection output groups
    NOG = D // 512          # out-projection output groups

    nc = bacc.Bacc("TRN2", target_bir_lowering=False, debug=False,
                   num_devices=N_CORES)

    xqt = nc.dram_tensor("xqt", [D, S], BF16, kind="ExternalInput")
    xkvt = nc.dram_tensor("xkvt", [D, T], BF16, kind="ExternalInput")
    wq = nc.dram_tensor("wq", [H, 128, NIC, DH], BF16, kind="ExternalInput")
    wk = nc.dram_tensor("wk", [H, 128, NIC, DH], BF16, kind="ExternalInput")
    wv = nc.dram_tensor("wv", [D, D], BF16, kind="ExternalInput")
    wo = nc.dram_tensor("wo", [D, D], BF16, kind="ExternalInput")
    bqd = nc.dram_tensor("bq", [D], F32, kind="ExternalInput")  # pre-scaled
    bkd = nc.dram_tensor("bk", [D], F32, kind="ExternalInput")
    bvd = nc.dram_tensor("bv", [D], F32, kind="ExternalInput")
    bod = nc.dram_tensor("bo", [D], F32, kind="ExternalInput")
    out = nc.dram_tensor("out", [S, D], F32, kind="ExternalOutput")

    xqt_v = xqt.ap().rearrange("(c p) s -> p c s", p=128)
    xkvt_v = xkvt.ap().rearrange("(c p) t -> p c t", p=128)
    wv_v = wv.ap().rearrange("(c p) (g dg) -> p c g dg", p=128, dg=512)
    wo_v = wo.ap().rearrange("(h p) (g dg) -> p h g dg", p=128, dg=512)
    out_v = out.ap().rearrange("(n p) (g dg) -> n p g dg", p=128, dg=512)

    from contextlib import ExitStack
    with tile.TileContext(nc) as tc, ExitStack() as es:
        const = es.enter_context(tc.tile_pool(name="const", bufs=1))
        persist = es.enter_context(tc.tile_pool(name="persist", bufs=1))
        psum = es.enter_context(tc.tile_pool(name="psum", bufs=1, space="PSUM"))

        ones_col = const.tile([128, 1], BF16)
        nc.gpsimd.memset(ones_col[:], 1.0)
        ones_row = const.tile([1, 512], BF16)
        nc.gpsimd.memset(ones_row[:], 1.0)
        bv_f32 = const.tile([1, D], F32, tag="bv_f32")
        bo_f32 = const.tile([1, D], F32, tag="bo_f32")
        nc.sync.dma_start(bv_f32[:], bvd.ap()[None, :])
        nc.sync.dma_start(bo_f32[:], bod.ap()[None, :])
        bv_sb = const.tile([1, D], BF16, tag="bv_sb")
        bo_sb = const.tile([1, D], BF16, tag="bo_sb")
        nc.vector.tensor_copy(bv_sb[:], bv_f32[:])
        nc.vector.tensor_copy(bo_sb[:], bo_f32[:])
        bq_col = const.tile([128, H], F32)
        bk_col = const.tile([128, H], F32)
        nc.sync.dma_start(bq_col[:], bqd.ap().rearrange("(h p) -> p h", p=128))
        nc.sync.dma_start(bk_col[:], bkd.ap().rearrange("(h p) -> p h", p=128))

        # persistent K^T and V (both bf16, SBUF-resident)
        kT = persist.tile([128, H, T], BF16)      # [dh, h, t]
        v_sb = persist.tile([128, NTC, D], BF16)  # [t%128, tc, d]

        # streaming pools that live across both phases (so phase-2
        # prefetch DMAs don't false-depend on phase-1 SBUF reuse)
        stream = es.enter_context(tc.tile_pool(name="stream", bufs=1))

        # ---------------- phase 1: K^T and V ----------------
        with tc.tile_pool(name="ph1", bufs=1) as ph1:
            xkvT = ph1.tile([128, NIC, T], BF16, tag="xkvT", bufs=1)
            wk_tiles = {}
            for h in range(2):  # prefetch first wk ahead of the big load
                wk_pre = ph1.tile([128, NIC, DH], BF16, tag="wk", bufs=2)
                nc.sync.dma_start(wk_pre[:], wk.ap()[h, :, :, :])
                wk_tiles[h] = wk_pre
            for c in range(NIC):
                nc.sync.dma_start(xkvT[:, c, :], xkvt_v[:, c, :])
            for h in range(H):
                if h in wk_tiles:
                    wk_h = wk_tiles.pop(h)
                else:
                    wk_h = ph1.tile([128, NIC, DH], BF16, tag="wk", bufs=2)
                    nc.sync.dma_start(wk_h[:], wk.ap()[h, :, :, :])
                for half in range(2):
                    pk = psum.tile([128, TH], F32, tag="M", bufs=2)
                    for c in range(NIC):
                        nc.tensor.matmul(
                            pk[:], wk_h[:, c, :],
                            xkvT[:, c, half * TH:(half + 1) * TH],
                            start=(c == 0), stop=(c == NIC - 1))
                    nc.scalar.activation(
                        kT[:, h, half * TH:(half + 1) * TH], pk[:],
                        AF.Identity, bias=bk_col[:, h:h + 1])
            for g in range(NVG):
                wv_g = ph1.tile([128, NIC, 512], BF16, tag="wv", bufs=2)
                nc.sync.dma_start(wv_g[:], wv_v[:, :, g, :])
                for tj in range(NTC):
                    pv = psum.tile([128, 512], F32, tag="M", bufs=2)
                    for c in range(NIC):
                        nc.tensor.matmul(
                            pv[:], xkvT[:, c, tj * 128:(tj + 1) * 128],
                            wv_g[:, c, :], start=(c == 0), stop=False)
                    nc.tensor.matmul(pv[:], ones_row[:, :128],
                                     bv_sb[:, g * 512:(g + 1) * 512],
                                     start=False, stop=True)
                    nc.scalar.activation(v_sb[:, tj, g * 512:(g + 1) * 512],
                                         pv[:], AF.Copy)

        # ---------------- phase 2: attention + out projection --------
        # Software-pipelined: the q-projection for head h+1 is emitted
        # BEFORE the attention of head h, so the in-order Tensor queue
        # always has independent matmuls to run while the softmax
        # chain (exp/evac/recip) catches up.
        with tc.tile_pool(name="ph2", bufs=1) as ph2:
            xqT_tiles = {}

            def get_xqT(sb):
                if sb not in xqT_tiles:
                    t = stream.tile([128, NIC, SB], BF16, tag="xqT", bufs=2)
                    nc.sync.dma_start(t[:],
                                      xqt_v[:, :, sb * SB:(sb + 1) * SB])
                    xqT_tiles[sb] = t
                return xqT_tiles[sb]

            def pq_group(sb, h):
                xq_t = get_xqT(sb)
                wq_h = stream.tile([128, NIC, DH], BF16, tag="wq", bufs=3)
                nc.sync.dma_start(wq_h[:], wq.ap()[h, :, :, :])
                pq = psum.tile([128, SB], F32, tag="M", bufs=2)
                for c in range(NIC):
                    nc.tensor.matmul(pq[:], wq_h[:, c, :], xq_t[:, c, :],
                                     start=(c == 0), stop=(c == NIC - 1))
                qT = ph2.tile([128, SB], BF16, tag="qT", bufs=2)
                nc.vector.tensor_scalar(qT[:], pq[:], bq_col[:, h:h + 1],
                                        None, mybir.AluOpType.add)
                return qT

            qT_next = pq_group(0, 0)
            for sb in range(NSB):
                ctx = ph2.tile([128, H, SB], BF16, tag="ctx", bufs=2)
                for h in range(H):
                    qT = qT_next
                    if h == 8 and sb + 1 < NSB:
                        get_xqT(sb + 1)          # prefetch next s-block
                    expsb = ph2.tile([128, NTC, SB], BF16, tag="exp", bufs=2)
                    pden = psum.tile([1, SB], F32, tag="D", bufs=1)
                    pctx = psum.tile([128, SB], F32, tag="C", bufs=1)

                    def sc_pair(tp):
                        psc = psum.tile([128, 2 * SB], F32, tag="B", bufs=2)
                        for u in range(2):
                            nc.tensor.matmul(
                                psc[:, u * SB:(u + 1) * SB],
                                kT[:, h, (2 * tp + u) * 128:
                                   (2 * tp + u + 1) * 128],
                                qT[:])
                        nc.scalar.activation(
                            expsb[:, 2 * tp:2 * tp + 2, :],
                            psc[:].rearrange("p (u s) -> p u s", u=2),
                            AF.Exp)

                    def dc(t):
                        nc.tensor.matmul(pctx[:],
                                         v_sb[:, t, h * DH:(h + 1) * DH],
                                         expsb[:, t, :],
                                         start=(t == 0), stop=(t == NTC - 1))
                        nc.tensor.matmul(pden[:], ones_col[:],
                                         expsb[:, t, :],
                                         start=(t == 0), stop=(t == NTC - 1))

                    # interleave: scores feed exp; den/ctx follow the
                    # exp stream; the next head's q-projection fills
                    # any remaining stall.
                    sc_pair(0)
                    sc_pair(1)
                    if h + 1 < H:
                        qT_next = pq_group(sb, h + 1)
                    elif sb + 1 < NSB:
                        qT_next = pq_group(sb + 1, 0)
                    dc(0); dc(1)
                    sc_pair(2)
                    dc(2); dc(3)
                    sc_pair(3)
                    dc(4); dc(5); dc(6); dc(7)

                    d1 = ph2.tile([1, SB], F32, tag="d1", bufs=2)
                    nc.scalar.activation(d1[:], pden[:], AF.Copy)
                    ctxu = ph2.tile([128, SB], BF16, tag="ctxu", bufs=2)
                    nc.vector.tensor_copy(ctxu[:], pctx[:])
                    recip = ph2.tile([1, SB], F32, tag="recip", bufs=2)
                    nc.vector.reciprocal_approx_fast(recip[:], d1[:])
                    rden = ph2.tile([128, SB], F32, tag="rden", bufs=2)
                    nc.gpsimd.partition_broadcast(rden[:], recip[:],
                                                  channels=128)
                    nc.vector.tensor_tensor(ctx[:, h, :], ctxu[:], rden[:],
                                            mybir.AluOpType.mult)
                # out projection: accumulate heads in PSUM, j-chunk pairs
                for g in range(NOG):
                    po = []
                    for _jp in range(NJ // 2):
                        po_jp = psum.tile([128, 1024], F32, tag="B", bufs=2)
                        po.append(po_jp)
                    for h in range(H):
                        wo_hg = ph2.tile([128, 512], BF16, tag="wo", bufs=2)
                        nc.sync.dma_start(wo_hg[:], wo_v[:, h, g, :])
                        for jp in range(NJ // 2):
                            for u in range(2):
                                nc.tensor.matmul(
                                    po[jp][:, u * 512:(u + 1) * 512],
                                    ctx[:, h, (2 * jp + u) * 128:
                                        (2 * jp + u + 1) * 128],
                                    wo_hg[:],
                                    start=(h == 0), stop=False)
                    for jp in range(NJ // 2):
                        for u in range(2):
                            nc.tensor.matmul(
                                po[jp][:, u * 512:(u + 1) * 512],
                                ones_row[:, :128],
                                bo_sb[:, g * 512:(g + 1) * 512],
                                start=False, stop=True)
                        o_sb = ph2.tile([128, 1024], F32, tag="osb", bufs=2)
                        if g % 2 == 0:
                            nc.scalar.activation(o_sb[:], po[jp][:], AF.Copy)
                        else:
                            nc.vector.tensor_copy(o_sb[:], po[jp][:])
                        for u in range(2):
                            nc.sync.dma_start(
                                out_v[sb * NJ + 2 * jp + u, :, g, :],
                                o_sb[:, u * 512:(u + 1) * 512])

    nc.compile()
    return nc


_NC_CACHE = {}


def _get_program(S=S_LOC, T=T_FULL, D=D_MODEL, H=NUM_HEADS):
    key = (S, T, D, H)
    if key not in _NC_CACHE:
        _NC_CACHE[key] = build_program(S, T, D, H)
    return _NC_CACHE[key]


def make_in_maps(query, key_value, Wq, bq, Wk, bk, Wv, bv, Wo, bo):
    f = np.float32
    import ml_dtypes
    bf = ml_dtypes.bfloat16
    D = Wq.shape[0]
    H = D // HEAD_DIM
    NIC = D // 128
    iscale = np.float32(1.0 / math.sqrt(HEAD_DIM))

    def per_head(W):
        # [h, p, c, dh] with value W[h*DH+dh, c*128+p]
        return np.ascontiguousarray(
            np.asarray(W, f).reshape(H, HEAD_DIM, NIC, 128)
            .transpose(0, 3, 2, 1))

    wq_h = per_head(np.asarray(Wq, f) * iscale).astype(bf)
    wk_h = per_head(Wk).astype(bf)
    shared = {
        "wq": wq_h,
        "wk": wk_h,
        "wv": np.ascontiguousarray(np.asarray(Wv).T).astype(bf),
        "wo": np.ascontiguousarray(np.asarray(Wo).T).astype(bf),
        "bq": np.asarray(bq, f) * iscale, "bk": np.asarray(bk, f),
        "bv": np.asarray(bv, f), "bo": np.asarray(bo, f),
    }
    n_batch = query.shape[0]
    halves = N_CORES // n_batch
    s_loc = query.shape[1] // halves
    in_maps = []
    kvt_cache = {}
    for c in range(N_CORES):
        b, hf = c // halves, c % halves
        if b not in kvt_cache:
            kvt_cache[b] = np.ascontiguousarray(
                np.asarray(key_value[b]).T).astype(bf)
        xq_slice = np.asarray(query[b, hf * s_loc:(hf + 1) * s_loc])
        in_maps.append({
            "xqt": np.ascontiguousarray(xq_slice.T).astype(bf),
            "xkvt": kvt_cache[b],
            **shared,
        })
    return in_maps


def run(inputs, trace=False, tmpdir=None):
    """Run the SPMD kernel; returns (full_output, BassKernelResults)."""
    query = np.asarray(inputs["query"])
    key_value = np.asarray(inputs["key_value"])
    nb, s_full, d = query.shape
    nc = _get_program(S=(nb * s_full) // N_CORES, T=key_value.shape[1], D=d,
                      H=d // HEAD_DIM)
    in_maps = make_in_maps(**inputs)
    res = run_bass_kernel_spmd(nc, in_maps, core_ids=list(range(N_CORES)),
                               trace=trace, tmpdir=tmpdir)
    halves = N_CORES // nb
    s_loc = s_full // halves
    out = np.empty((nb, s_full, d), np.float32)
    for c in range(N_CORES):
        b, hf = c // halves, c % halves
        out[b, hf * s_loc:(hf + 1) * s_loc] = res.results[c]["out"]
    return out, res


def kernel(**inputs) -> np.ndarray:
    out, _ = run(inputs, trace=False)
    return out


# revision 14
# speedup vs baseline: 1.2727x; 1.2727x over previous
"""Cross-attention LLM block on 8 Trainium2 NeuronCores.

Sharding: core c handles batch b = c//2 and query-row half h = c%2
(2048 of the 4096 query rows of that batch), for ALL 16 heads.
K/V projections for a batch are computed redundantly by the two cores
sharing that batch (~12% extra FLOPs) so no cross-core communication
is needed; the host only slices/transposes inputs and concatenates
outputs.

v2 design (vs v1): the host pre-transposes activations to bf16
(xqT=[D,S], xkvT=[D,T]) so the device does zero PE transposes; V
lives in SBUF (no DRAM round-trip); weights use per-head-contiguous
host layouts for line-rate DMA; all PSUM evacuations ride the Scalar
engine (activation Identity/Copy, one table set) fusing bias adds, so
the Vector engine only does the softmax reciprocal+scale; exp is
batched over [128,1024] PSUM pairs. PSUM tags: M(pq,2) B(score
pairs + out pairs, 2x2 banks) C(pctx,1) D(pden,1) = 8 banks, each
evacuated within ~720ns so matmuls never stall.

Per-core dataflow (all matmuls bf16, N=512):
  phase 1: kT[dh,h,t] = sum_c wk_h_c.T @ xkvT_c   (+bk via ACT evac)
           v[t,tc,d]  = sum_c xkvT_c.T @ wv_c     (+bv rank-1 MM)
  phase 2, per s-block of 512 rows, per head:
           qT[dh,s]  = sum_c wq_h_c.T @ xqT_c     (+bq*s via ACT evac)
           psc[t,s]  = kT_ht.T @ qT   (pairs of t-chunks share a
                       2-bank PSUM tile; one exp per pair)
           e = exp(psc)               (ACT, bf16 -> SBUF)
           pden[1,s] += ones.T @ e ; pctx[dh,s] += v_ht.T @ e
           ACT: pden->d1, pctx->ctxu  (frees banks fast)
           DVE: recip(d1); GpSimd: broadcast; DVE: ctx = ctxu*rden
  out[s128,dg512] = sum_h ctx_hj.T @ wo_hg  (+bo rank-1, PSUM pairs)
"""

import math
import sys

for _p in ("/opt/trn_rl_repo",):
    if _p not in sys.path:
        sys.path.append(_p)

import numpy as np

import concourse.bass as bass
import concourse.mybir as mybir
import concourse.tile as tile
from concourse import bacc
from concourse.bass_utils import run_bass_kernel_spmd

F32 = mybir.dt.float32
BF16 = mybir.dt.bfloat16
AF = mybir.ActivationFunctionType

# full-problem dims
B, S_FULL, T_FULL, D_MODEL, NUM_HEADS = 4, 4096, 1024, 2048, 16
HEAD_DIM = 128
N_CORES = 8
S_LOC = (B * S_FULL) // N_CORES  # 2048 query rows per core


def build_program(S=S_LOC, T=T_FULL, D=D_MODEL, H=NUM_HEADS):
    """Build + compile the single-core program (SPMD across 8 cores)."""
    DH = HEAD_DIM
    NIC = D // 128          # contraction chunks
    TH = T // 2             # t-half (512 cols per K psum)
    NTC = T // 128          # t-chunks
    SB = min(512, S)        # s-block
    NSB = S // SB
    NJ = SB // 128          # 128-row subchunks per s-block
    NVG = D // 512          # v-projection output groups
    NOG = D // 512          # out-projection output groups

    nc = bacc.Bacc("TRN2", target_bir_lowering=False, debug=False,
                   num_devices=N_CORES)

    xqt = nc.dram_tensor("xqt", [D, S], BF16, kind="ExternalInput")
    xkvt = nc.dram_tensor("xkvt", [D, T], BF16, kind="ExternalInput")
    wq = nc.dram_tensor("wq", [H, 128, NIC, DH], BF16, kind="ExternalInput")
    wk = nc.dram_tensor("wk", [H, 128, NIC, DH], BF16, kind="ExternalInput")
    wv = nc.dram_tensor("wv", [D, D], BF16, kind="ExternalInput")
    wo = nc.dram_tensor("wo", [D, D], BF16, kind="ExternalInput")
    bqd = nc.dram_tensor("bq", [D], F32, kind="ExternalInput")  # pre-scaled
    bkd = nc.dram_tensor("bk", [D], F32, kind="ExternalInput")
    bvd = nc.dram_tensor("bv", [D], F32, kind="ExternalInput")
    bod = nc.dram_tensor("bo", [D], F32, kind="ExternalInput")
    out = nc.dram_tensor("out", [S, D], F32, kind="ExternalOutput")

    xqt_v = xqt.ap().rearrange("(c p) s -> p c s", p=128)
    xkvt_v = xkvt.ap().rearrange("(c p) t -> p c t", p=128)
    wv_v = wv.ap().rearrange("(c p) (g dg) -> p c g dg", p=128, dg=512)
    wo_v = wo.ap().rearrange("(h p) (g dg) -> p h g dg", p=128, dg=512)
    out_v = out.ap().rearrange("(n p) (g dg) -> n p g dg", p=128, dg=512)

    from contextlib import ExitStack
    with tile.TileContext(nc) as tc, ExitStack() as es:
        const = es.enter_context(tc.tile_pool(name="const", bufs=1))
        persist = es.enter_context(tc.tile_pool(name="persist", bufs=1))
        psum = es.enter_context(tc.tile_pool(name="psum", bufs=1, space="PSUM"))

        ones_col = const.tile([128, 1], BF16)
        nc.gpsimd.memset(ones_col[:], 1.0)
        ones_row = const.tile([1, 512], BF16)
        nc.gpsimd.memset(ones_row[:], 1.0)
        bv_sb = const.tile([1, D], BF16, tag="bv_sb")
        bo_sb = const.tile([1, D], BF16, tag="bo_sb")
        with tc.tile_pool(name="biasstage", bufs=1) as bstage:
            bv_f32 = bstage.tile([1, D], F32, tag="bv_f32")
            bo_f32 = bstage.tile([1, D], F32, tag="bo_f32")
            nc.sync.dma_start(bv_f32[:], bvd.ap()[None, :])
            nc.sync.dma_start(bo_f32[:], bod.ap()[None, :])
            nc.vector.tensor_copy(bv_sb[:], bv_f32[:])
            nc.vector.tensor_copy(bo_sb[:], bo_f32[:])
        bq_col = const.tile([128, H], F32)
        bk_col = const.tile([128, H], F32)
        nc.sync.dma_start(bq_col[:], bqd.ap().rearrange("(h p) -> p h", p=128))
        nc.sync.dma_start(bk_col[:], bkd.ap().rearrange("(h p) -> p h", p=128))

        # persistent K^T and V (both bf16, SBUF-resident)
        kT = persist.tile([128, H, T], BF16)      # [dh, h, t]
        v_sb = persist.tile([128, NTC, D], BF16)  # [t%128, tc, d]

        # streaming pools that live across both phases (so phase-2
        # prefetch DMAs don't false-depend on phase-1 SBUF reuse)
        stream = es.enter_context(tc.tile_pool(name="stream", bufs=1))

        # ---------------- phase 1: K^T and V ----------------
        with tc.tile_pool(name="ph1", bufs=1) as ph1:
            xkvT = ph1.tile([128, NIC, T], BF16, tag="xkvT", bufs=1)
            wk_tiles = {}
            for h in range(2):  # prefetch first wk ahead of the big load
                wk_pre = ph1.tile([128, NIC, DH], BF16, tag="wk", bufs=2)
                nc.sync.dma_start(wk_pre[:], wk.ap()[h, :, :, :])
                wk_tiles[h] = wk_pre
            for c in range(NIC):
                nc.sync.dma_start(xkvT[:, c, :], xkvt_v[:, c, :])
            for h in range(H):
                if h in wk_tiles:
                    wk_h = wk_tiles.pop(h)
                else:
                    wk_h = ph1.tile([128, NIC, DH], BF16, tag="wk", bufs=2)
                    nc.sync.dma_start(wk_h[:], wk.ap()[h, :, :, :])
                for half in range(2):
                    pk = psum.tile([128, TH], F32, tag="M", bufs=2)
                    for c in range(NIC):
                        nc.tensor.matmul(
                            pk[:], wk_h[:, c, :],
                            xkvT[:, c, half * TH:(half + 1) * TH],
                            start=(c == 0), stop=(c == NIC - 1))
                    nc.scalar.activation(
                        kT[:, h, half * TH:(half + 1) * TH], pk[:],
                        AF.Identity, bias=bk_col[:, h:h + 1])
            for g in range(NVG):
                wv_g = ph1.tile([128, NIC, 512], BF16, tag="wv", bufs=2)
                nc.sync.dma_start(wv_g[:], wv_v[:, :, g, :])
                for tj in range(NTC):
                    pv = psum.tile([128, 512], F32, tag="M", bufs=2)
                    for c in range(NIC):
                        nc.tensor.matmul(
                            pv[:], xkvT[:, c, tj * 128:(tj + 1) * 128],
                            wv_g[:, c, :], start=(c == 0), stop=False)
                    nc.tensor.matmul(pv[:], ones_row[:, :128],
                                     bv_sb[:, g * 512:(g + 1) * 512],
                                     start=False, stop=True)
                    nc.scalar.activation(v_sb[:, tj, g * 512:(g + 1) * 512],
                                         pv[:], AF.Copy)

        # ---------------- phase 2: attention + out projection --------
        # Software-pipelined: the q-projection for head h+1 is emitted
        # BEFORE the attention of head h, so the in-order Tensor queue
        # always has independent matmuls to run while the softmax
        # chain (exp/evac/recip) catches up.
        with tc.tile_pool(name="ph2", bufs=1) as ph2:
            xqT_tiles = {}

            def get_xqT(sb):
                if sb not in xqT_tiles:
                    t = stream.tile([128, NIC, SB], BF16, tag="xqT", bufs=2)
                    nc.sync.dma_start(t[:],
                                      xqt_v[:, :, sb * SB:(sb + 1) * SB])
                    xqT_tiles[sb] = t
                return xqT_tiles[sb]

            def pq_group(sb, h):
                xq_t = get_xqT(sb)
                wq_h = stream.tile([128, NIC, DH], BF16, tag="wq", bufs=3)
                nc.sync.dma_start(wq_h[:], wq.ap()[h, :, :, :])
                pq = psum.tile([128, SB], F32, tag="M", bufs=2)
                for c in range(NIC):
                    nc.tensor.matmul(pq[:], wq_h[:, c, :], xq_t[:, c, :],
                                     start=(c == 0), stop=(c == NIC - 1))
                qT = ph2.tile([128, SB], BF16, tag="qT", bufs=2)
                nc.vector.tensor_scalar(qT[:], pq[:], bq_col[:, h:h + 1],
                                        None, mybir.AluOpType.add)
                return qT

            qT_next = pq_group(0, 0)
            for sb in range(NSB):
                ctx = ph2.tile([128, H, SB], BF16, tag="ctx", bufs=2)
                for h in range(H):
                    qT = qT_next
                    if h == 8 and sb + 1 < NSB:
                        get_xqT(sb + 1)          # prefetch next s-block
                    expsb = ph2.tile([128, NTC, SB], BF16, tag="exp", bufs=2)
                    pden = psum.tile([1, SB], F32, tag="D", bufs=1)
                    pctx = psum.tile([128, SB], F32, tag="C", bufs=1)

                    def sc_pair(tp):
                        psc = psum.tile([128, 2 * SB], F32, tag="B", bufs=2)
                        for u in range(2):
                            nc.tensor.matmul(
                                psc[:, u * SB:(u + 1) * SB],
                                kT[:, h, (2 * tp + u) * 128:
                                   (2 * tp + u + 1) * 128],
                                qT[:])
                        nc.scalar.activation(
                            expsb[:, 2 * tp:2 * tp + 2, :],
                            psc[:].rearrange("p (u s) -> p u s", u=2),
                            AF.Exp)

                    def dc(t):
                        nc.tensor.matmul(pctx[:],
                                         v_sb[:, t, h * DH:(h + 1) * DH],
                                         expsb[:, t, :],
                                         start=(t == 0), stop=(t == NTC - 1))
                        nc.tensor.matmul(pden[:], ones_col[:],
                                         expsb[:, t, :],
                                         start=(t == 0), stop=(t == NTC - 1))

                    # interleave: scores feed exp; den/ctx follow the
                    # exp stream; the next head's q-projection fills
                    # any remaining stall.
                    sc_pair(0)
                    sc_pair(1)
                    if h + 1 < H:
                        qT_next = pq_group(sb, h + 1)
                    elif sb + 1 < NSB:
                        qT_next = pq_group(sb + 1, 0)
                    dc(0); dc(1)
                    sc_pair(2)
                    dc(2); dc(3)
                    sc_pair(3)
                    dc(4); dc(5); dc(6); dc(7)

                    d1 = ph2.tile([1, SB], F32, tag="d1", bufs=2)
                    nc.scalar.activation(d1[:], pden[:], AF.Copy)
                    ctxu = ph2.tile([128, SB], BF16, tag="ctxu", bufs=2)
                    nc.vector.tensor_copy(ctxu[:], pctx[:])
                    recip = ph2.tile([1, SB], F32, tag="recip", bufs=2)
                    nc.vector.reciprocal_approx_fast(recip[:], d1[:])
                    rden = ph2.tile([128, SB], F32, tag="rden", bufs=2)
                    nc.gpsimd.partition_broadcast(rden[:], recip[:],
                                                  channels=128)
                    nc.vector.tensor_tensor(ctx[:, h, :], ctxu[:], rden[:],
                                            mybir.AluOpType.mult)
                # out projection: accumulate heads in PSUM, j-chunk pairs.
                # wo loads are 1MB half-head groups so po matmuls are not
                # paced by per-DMA completion latency.
                for g in range(NOG):
                    po = []
                    for _jp in range(NJ // 2):
                        po_jp = psum.tile([128, 1024], F32, tag="B", bufs=2)
                        po.append(po_jp)
                    for hh in range(2):
                        wo_hg = ph2.tile([128, H // 2, 512], BF16, tag="wo",
                                         bufs=2)
                        nc.sync.dma_start(
                            wo_hg[:],
                            wo_v[:, hh * (H // 2):(hh + 1) * (H // 2), g, :])
                        for hi in range(H // 2):
                            h = hh * (H // 2) + hi
                            for jp in range(NJ // 2):
                                for u in range(2):
                                    nc.tensor.matmul(
                                        po[jp][:, u * 512:(u + 1) * 512],
                                        ctx[:, h, (2 * jp + u) * 128:
                                            (2 * jp + u + 1) * 128],
                                        wo_hg[:, hi, :],
                                        start=(h == 0), stop=False)
                    for jp in range(NJ // 2):
                        for u in range(2):
                            nc.tensor.matmul(
                                po[jp][:, u * 512:(u + 1) * 512],
                                ones_row[:, :128],
                                bo_sb[:, g * 512:(g + 1) * 512],
                                start=False, stop=True)
                        o_sb = ph2.tile([128, 1024], F32, tag="osb", bufs=2)
                        if g % 2 == 0:
                            nc.scalar.activation(o_sb[:], po[jp][:], AF.Copy)
                        else:
                            nc.vector.tensor_copy(o_sb[:], po[jp][:])
                        for u in range(2):
                            nc.sync.dma_start(
                                out_v[sb * NJ + 2 * jp + u, :, g, :],
                                o_sb[:, u * 512:(u + 1) * 512])

    nc.compile()
    return nc


_NC_CACHE = {}


def _get_program(S=S_LOC, T=T_FULL, D=D_MODEL, H=NUM_HEADS):
    key = (S, T, D, H)
    if key not in _NC_CACHE:
        _NC_CACHE[key] = build_program(S, T, D, H)
    return _NC_CACHE[key]


def make_in_maps(query, key_value, Wq, bq, Wk, bk, Wv, bv, Wo, bo):
    f = np.float32
    import ml_dtypes
    bf = ml_dtypes.bfloat16
    D = Wq.shape[0]
    H = D // HEAD_DIM
    NIC = D // 128
    iscale = np.float32(1.0 / math.sqrt(HEAD_DIM))

    def per_head(W):
        # [h, p, c, dh] with value W[h*DH+dh, c*128+p]
        return np.ascontiguousarray(
            np.asarray(W, f).reshape(H, HEAD_DIM, NIC, 128)
            .transpose(0, 3, 2, 1))

    wq_h = per_head(np.asarray(Wq, f) * iscale).astype(bf)
    wk_h = per_head(Wk).astype(bf)
    shared = {
        "wq": wq_h,
        "wk": wk_h,
        "wv": np.ascontiguousarray(np.asarray(Wv).T).astype(bf),
        "wo": np.ascontiguousarray(np.asarray(Wo).T).astype(bf),
        "bq": np.asarray(bq, f) * iscale, "bk": np.asarray(bk, f),
        "bv": np.asarray(bv, f), "bo": np.asarray(bo, f),
    }
    n_batch = query.shape[0]
    halves = N_CORES // n_batch
    s_loc = query.shape[1] // halves
    in_maps = []
    kvt_cache = {}
    for c in range(N_CORES):
        b, hf = c // halves, c % halves
        if b not in kvt_cache:
            kvt_cache[b] = np.ascontiguousarray(
                np.asarray(key_value[b]).T).astype(bf)
        xq_slice = np.asarray(query[b, hf * s_loc:(hf + 1) * s_loc])
        in_maps.append({
            "xqt": np.ascontiguousarray(xq_slice.T).astype(bf),
            "xkvt": kvt_cache[b],
            **shared,
        })
    return in_maps


def run(inputs, trace=False, tmpdir=None):
    """Run the SPMD kernel; returns (full_output, BassKernelResults)."""
    query = np.asarray(inputs["query"])
    key_value = np.asarray(inputs["key_value"])
    nb, s_full, d = query.shape
    nc = _get_program(S=(nb * s_full) // N_CORES, T=key_value.shape[1], D=d,
                      H=d // HEAD_DIM)
    in_maps = make_in_maps(**inputs)
    res = run_bass_kernel_spmd(nc, in_maps, core_ids=list(range(N_CORES)),
                               trace=trace, tmpdir=tmpdir)
    halves = N_CORES // nb
    s_loc = s_full // halves
    out = np.empty((nb, s_full, d), np.float32)
    for c in range(N_CORES):
        b, hf = c // halves, c % halves
        out[b, hf * s_loc:(hf + 1) * s_loc] = res.results[c]["out"]
    return out, res


def kernel(**inputs) -> np.ndarray:
    out, _ = run(inputs, trace=False)
    return out
